# revision 18
# baseline (speedup 1.0000x reference)
"""v7: 2 pairs per dma_start (halves per-engine DMA semaphore-packet and
issue overhead), fp32 no-cast loads, ACT casts, 2-pair MM stagger."""

import sys

for _p in ("/opt/trn_rl_repo", "/root/.axon_site/_ro/trn_rl_repo"):
    if _p not in sys.path:
        sys.path.insert(0, _p)

import numpy as np

B, TOPK, L, D = 8, 64, 256, 768
N_CORES = 8
PAIRS = 64
P = 128
DCHUNKS = D // P  # 6
LCHUNKS = L // P  # 2
G = 2  # pairs per dma group
NGROUPS = PAIRS // G

LOAD = "swdge"  # "swdge" | "mixed" (ctx swdge, ent hwdge)
XBUFS = 5  # of [128, G*4, 768] f32 tiles
WARM_MMS = 20

_cache = {}


def _build():
    import concourse.bass as bass
    import concourse.mybir as mybir
    from concourse import bacc
    from concourse.tile import TileContext
    from concourse.masks import make_identity

    nc = bacc.Bacc(
        "TRN2", target_bir_lowering=False, debug=False, num_devices=N_CORES
    )
    x = nc.dram_tensor(
        "x", [PAIRS, 2, L, D], mybir.dt.float32, kind="ExternalInput"
    ).ap()
    out = nc.dram_tensor(
        "out", [1, PAIRS], mybir.dt.float32, kind="ExternalOutput"
    ).ap()
    bf16 = mybir.dt.bfloat16
    f32 = mybir.dt.float32

    # group view: pair = G*g + gp; partition p <- rows 2p, 2p+1 per slab
    xg = x.rearrange("(g gp) s (p two) d -> g gp s p (two d)", gp=G, p=P)

    with TileContext(nc) as tc:
        with (
            tc.tile_pool(name="const", bufs=1) as cpool,
            tc.tile_pool(name="xload", bufs=XBUFS) as xpool,
            tc.tile_pool(name="xcast", bufs=3) as bpool,
            tc.tile_pool(name="tpose", bufs=4) as tpool,
            tc.tile_pool(name="ppose", bufs=4, space="PSUM") as ppool,
            tc.tile_pool(name="pmm", bufs=3, space="PSUM") as mpool,
            tc.tile_pool(name="pfin", bufs=1, space="PSUM") as fpool,
        ):
            ident = cpool.tile([P, P], bf16)
            ones = cpool.tile([P, 1], f32)
            RM = cpool.tile([P, 2 * PAIRS], f32)
            wsb = cpool.tile([P, 256], bf16)

            nc.gpsimd.memset(wsb, 0.0)
            wps = fpool.tile([P, 256], f32, tag="fin", name="wps")
            for _ in range(WARM_MMS):
                nc.tensor.matmul(wps, wsb[:, :P], wsb, start=True, stop=True)

            fin = fpool.tile([1, 2 * PAIRS], f32, tag="fin", name="fin")

            def emit_mm(pair, T):
                ps = mpool.tile([P, LCHUNKS, 2 * P], f32)
                for lc in range(LCHUNKS):
                    for dc in range(DCHUNKS):
                        off = (dc * 2 + lc) * P
                        nc.tensor.matmul(
                            ps[:, lc],
                            T[:, off : off + P],
                            T[:, 1536 + dc * 2 * P : 1536 + (dc + 1) * 2 * P],
                            start=(dc == 0),
                            stop=(dc == DCHUNKS - 1),
                        )
                nc.vector.reduce_max(
                    RM[:, 2 * pair : 2 * pair + 2], ps, axis=mybir.AxisListType.X
                )
                if pair == PAIRS // 2 - 1:
                    nc.tensor.matmul(
                        fin[:, :PAIRS], ones, RM[:, :PAIRS], start=True, stop=True
                    )

            pend = []
            for g in range(NGROUPS):
                # X: [p, gp, c, d]; c 0:2 ctx rows, 2:4 ent rows
                X = xpool.tile([P, G, 4, D], f32, tag="X", name="X")
                Xb = bpool.tile([P, G, 4, D], bf16, tag="Xb", name="Xb")
                dma_ctx = nc.gpsimd
                dma_ent = nc.gpsimd if LOAD == "swdge" else nc.sync
                # one dma per slab type covering both pairs of the group
                dma_ctx.dma_start(
                    X[:, :, 0:2, :].rearrange("p gp c d -> p gp (c d)"),
                    xg[g, :, 0].rearrange("gp p cd -> p gp cd"),
                )
                dma_ent.dma_start(
                    X[:, :, 2:4, :].rearrange("p gp c d -> p gp (c d)"),
                    xg[g, :, 1].rearrange("gp p cd -> p gp cd"),
                )
                if g == 0:
                    make_identity(nc, ident)
                    nc.gpsimd.memset(ones, 1.0)

                for gp in range(G):
                    q = G * g + gp
                    if len(pend) > 2:
                        emit_mm(*pend.pop(0))
                    # fp32 -> bf16 on ACT, one [128,768] chunk per row-slot:
                    # finer chunks let the first transposes start ~0.8us
                    # earlier and halve ACT's FIFO blocking granularity
                    for c in range(4):
                        nc.scalar.copy(Xb[:, gp, c, :], X[:, gp, c, :])
                    T = tpool.tile([P, 2 * 1536], bf16, tag="T", name="T")
                    for jj in range(3):
                        psb = ppool.tile([P, 1024], bf16, tag="psb", name="psb")
                        for slot in range(8):
                            j = jj * 8 + slot
                            t, rem = divmod(j, 12)
                            dc, lc = divmod(rem, 2)
                            c = t * 2 + lc
                            nc.tensor.transpose(
                                psb[:, slot * P : (slot + 1) * P],
                                Xb[:, gp, c, dc * P : (dc + 1) * P],
                                ident,
                            )
                        dst = T[:, jj * 1024 : (jj + 1) * 1024]
                        nc.vector.tensor_copy(dst, psb)
                    pend.append((q, T))

            for item in pend:
                emit_mm(*item)

            nc.tensor.matmul(
                fin[:, PAIRS:], ones, RM[:, PAIRS:], start=True, stop=True
            )
            fsb = cpool.tile([1, 2 * PAIRS], f32)
            nc.vector.tensor_copy(fsb, fin)
            osb = cpool.tile([1, PAIRS], f32)
            fsb2 = fsb.rearrange("p (n two) -> p n two", two=2)
            nc.vector.tensor_tensor(
                osb, fsb2[:, :, 0], fsb2[:, :, 1], op=mybir.AluOpType.add
            )
            nc.sync.dma_start(out, osb)

    nc.compile()
    return nc


def _get_nc():
    if "nc" not in _cache:
        _cache["nc"] = _build()
    return _cache["nc"]


def run(context, trace=False, tmpdir=None):
    from concourse import bass_utils

    nc = _get_nc()
    context = np.ascontiguousarray(np.asarray(context, dtype=np.float32))
    assert context.shape == (B, TOPK, 2, L, D), context.shape
    in_maps = [{"x": context[c]} for c in range(N_CORES)]
    res = bass_utils.run_bass_kernel_spmd(
        nc, in_maps, core_ids=list(range(N_CORES)), trace=trace, tmpdir=tmpdir
    )
    out = np.concatenate(
        [res.results[c]["out"].reshape(1, PAIRS) for c in range(N_CORES)],
        axis=0,
    ).astype(np.float32)
    return out, res


def kernel(context):
    out, _ = run(context, trace=False)
    return out


# revision 19
# speedup vs baseline: 1.1280x; 1.1280x over previous
"""v7: 2 pairs per dma_start (halves per-engine DMA semaphore-packet and
issue overhead), fp32 no-cast loads, ACT casts, 2-pair MM stagger."""

import sys

for _p in ("/opt/trn_rl_repo", "/root/.axon_site/_ro/trn_rl_repo"):
    if _p not in sys.path:
        sys.path.insert(0, _p)

import numpy as np

B, TOPK, L, D = 8, 64, 256, 768
N_CORES = 8
PAIRS = 64
P = 128
DCHUNKS = D // P  # 6
LCHUNKS = L // P  # 2
G = 2  # pairs per dma group
NGROUPS = PAIRS // G

LOAD = "swdge"  # "swdge" | "mixed" (ctx swdge, ent hwdge)
XBUFS = 5  # of [128, G*4, 768] f32 tiles
WARM_MMS = 20

_cache = {}


def _build():
    import concourse.bass as bass
    import concourse.mybir as mybir
    from concourse import bacc
    from concourse.tile import TileContext
    from concourse.masks import make_identity

    nc = bacc.Bacc(
        "TRN2", target_bir_lowering=False, debug=False, num_devices=N_CORES
    )
    x = nc.dram_tensor(
        "x", [PAIRS, 2, L, D], mybir.dt.float32, kind="ExternalInput"
    ).ap()
    out = nc.dram_tensor(
        "out", [1, PAIRS], mybir.dt.float32, kind="ExternalOutput"
    ).ap()
    bf16 = mybir.dt.bfloat16
    f32 = mybir.dt.float32

    # group view: pair = G*g + gp; partition p <- rows 2p, 2p+1 per slab
    xg = x.rearrange("(g gp) s (p two) d -> g gp s p (two d)", gp=G, p=P)

    with TileContext(nc) as tc:
        with (
            tc.tile_pool(name="const", bufs=1) as cpool,
            tc.tile_pool(name="xload", bufs=XBUFS) as xpool,
            tc.tile_pool(name="xcast", bufs=3) as bpool,
            tc.tile_pool(name="tpose", bufs=4) as tpool,
            tc.tile_pool(name="ppose", bufs=4, space="PSUM") as ppool,
            tc.tile_pool(name="pmm", bufs=3, space="PSUM") as mpool,
            tc.tile_pool(name="pfin", bufs=1, space="PSUM") as fpool,
        ):
            ident = cpool.tile([P, P], bf16)
            ones = cpool.tile([P, 1], f32)
            RM = cpool.tile([P, 2 * PAIRS], f32)
            wsb = cpool.tile([P, 256], bf16)

            nc.gpsimd.memset(wsb, 0.0)
            wps = fpool.tile([P, 256], f32, tag="fin", name="wps")
            for _ in range(WARM_MMS):
                nc.tensor.matmul(wps, wsb[:, :P], wsb, start=True, stop=True)

            fin = fpool.tile([1, 2 * PAIRS], f32, tag="fin", name="fin")

            def emit_mm(pair, T):
                ps = mpool.tile([P, LCHUNKS, 2 * P], f32)
                for lc in range(LCHUNKS):
                    for dc in range(DCHUNKS):
                        off = (dc * 2 + lc) * P
                        nc.tensor.matmul(
                            ps[:, lc],
                            T[:, off : off + P],
                            T[:, 1536 + dc * 2 * P : 1536 + (dc + 1) * 2 * P],
                            start=(dc == 0),
                            stop=(dc == DCHUNKS - 1),
                        )
                nc.vector.reduce_max(
                    RM[:, 2 * pair : 2 * pair + 2], ps, axis=mybir.AxisListType.X
                )
                if pair == PAIRS // 2 - 1:
                    nc.tensor.matmul(
                        fin[:, :PAIRS], ones, RM[:, :PAIRS], start=True, stop=True
                    )

            pend = []
            for g in range(NGROUPS):
                # X: [p, gp, c, d]; c 0:2 ctx rows, 2:4 ent rows
                X = xpool.tile([P, G, 4, D], f32, tag="X", name="X")
                Xb = bpool.tile([P, G, 4, D], bf16, tag="Xb", name="Xb")
                dma_ctx = nc.gpsimd
                dma_ent = nc.gpsimd if LOAD == "swdge" else nc.sync
                # one dma per slab type covering both pairs of the group
                dma_ctx.dma_start(
                    X[:, :, 0:2, :].rearrange("p gp c d -> p gp (c d)"),
                    xg[g, :, 0].rearrange("gp p cd -> p gp cd"),
                )
                dma_ent.dma_start(
                    X[:, :, 2:4, :].rearrange("p gp c d -> p gp (c d)"),
                    xg[g, :, 1].rearrange("gp p cd -> p gp cd"),
                )
                if g == 0:
                    make_identity(nc, ident)
                    nc.gpsimd.memset(ones, 1.0)

                for gp in range(G):
                    q = G * g + gp
                    if len(pend) > 2:
                        emit_mm(*pend.pop(0))
                    # fp32 -> bf16 on ACT, one chunk per (pair, slab)
                    for s in range(2):
                        nc.scalar.copy(
                            Xb[:, gp, 2 * s : 2 * s + 2, :].rearrange(
                                "p c d -> p (c d)"
                            ),
                            X[:, gp, 2 * s : 2 * s + 2, :].rearrange(
                                "p c d -> p (c d)"
                            ),
                        )
                    T = tpool.tile([P, 2 * 1536], bf16, tag="T", name="T")
                    for jj in range(3):
                        psb = ppool.tile([P, 1024], bf16, tag="psb", name="psb")
                        for slot in range(8):
                            j = jj * 8 + slot
                            t, rem = divmod(j, 12)
                            dc, lc = divmod(rem, 2)
                            c = t * 2 + lc
                            nc.tensor.transpose(
                                psb[:, slot * P : (slot + 1) * P],
                                Xb[:, gp, c, dc * P : (dc + 1) * P],
                                ident,
                            )
                        dst = T[:, jj * 1024 : (jj + 1) * 1024]
                        nc.vector.tensor_copy(dst, psb)
                    pend.append((q, T))

            for item in pend:
                emit_mm(*item)

            nc.tensor.matmul(
                fin[:, PAIRS:], ones, RM[:, PAIRS:], start=True, stop=True
            )
            fsb = cpool.tile([1, 2 * PAIRS], f32)
            nc.vector.tensor_copy(fsb, fin)
            osb = cpool.tile([1, PAIRS], f32)
            fsb2 = fsb.rearrange("p (n two) -> p n two", two=2)
            nc.vector.tensor_tensor(
                osb, fsb2[:, :, 0], fsb2[:, :, 1], op=mybir.AluOpType.add
            )
            nc.sync.dma_start(out, osb)

    nc.compile()
    return nc


def _get_nc():
    if "nc" not in _cache:
        _cache["nc"] = _build()
    return _cache["nc"]


def run(context, trace=False, tmpdir=None):
    from concourse import bass_utils

    nc = _get_nc()
    context = np.ascontiguousarray(np.asarray(context, dtype=np.float32))
    assert context.shape == (B, TOPK, 2, L, D), context.shape
    in_maps = [{"x": context[c]} for c in range(N_CORES)]
    res = bass_utils.run_bass_kernel_spmd(
        nc, in_maps, core_ids=list(range(N_CORES)), trace=trace, tmpdir=tmpdir
    )
    out = np.concatenate(
        [res.results[c]["out"].reshape(1, PAIRS) for c in range(N_CORES)],
        axis=0,
    ).astype(np.float32)
    return out, res


def kernel(context):
    out, _ = run(context, trace=False)
    return out
